# revision 1
# baseline (speedup 1.0000x reference)
"""Trainium2 Bass kernel for the CAM (channel attention) module.

Computes, per batch element b:
    q = x[b].reshape(C, N)                      # C=512, N=4096
    E = q @ q.T                                 # C x C  (symmetric)
    att = softmax(rowmax(E) - E, axis=-1)       # == softmax(-E) row-wise
    out = gamma * (att @ q) + x[b]

Sharding: data-parallel over batch. 16 batch elements -> 2 per NeuronCore
across 8 cores. gamma replicated. No collectives.

Per-core kernel strategy (per batch element):
  1. DMA q into SBUF in natural layout qnat[c_part, n_free] (fp32, exact bits
     are reused for the +x residual, so this tile is never rounded).
  2. Build qT[n_part, c_free] with 128 PE [128x128] transposes; 4 transposes
     share one PSUM bank so a single [128,512] DVE copy drains them (4x fewer
     DVE ops). qT is stored as float32r: the DVE copy rounds, satisfying the
     fp32r-producer rule, and the energy matmul then runs at full PE rate
     (1 cycle/row) instead of fp32's 1/4 rate.
  3. E tiles [128, 512] accumulate in PSUM via fp32r matmuls.
  4. Column-oriented softmax avoids transposing the attention matrix:
     att_T[d, c] = exp(min_c - E[d, c]) / R_c with R_c = sum_d exp(...).
     E is symmetric so min_c (row mins) equals the column-min vector; the
     stored E tile read with d on partitions is already att_T-oriented.
     exp argument <= 0 always, so no overflow; R is clamped before the
     reciprocal so no NaN is possible.
  5. U = exp(min_c - E) in bf16 is the stationary operand of the value
     matmul against a bf16 copy of q (cast on the idle scalar engine).
     gamma/R_c (per output partition) and the +x residual are fused into one
     DVE scalar_tensor_tensor per output chunk; x enters only here, in exact
     fp32, so for gamma == 0 the kernel output is bit-exact x.
"""

import sys

import numpy as np

_REPO = "/opt/trn_rl_repo"
if _REPO not in sys.path:
    sys.path.insert(0, _REPO)

B_TOTAL, C, H, W = 16, 512, 64, 64
N = H * W          # 4096
NCORES = 8
B = B_TOTAL // NCORES  # batches per core = 2
CT = C // 128      # 4 c-tiles
NT = N // 128      # 32 n-tiles
NCH = N // 512     # 8 output column chunks

_cache = {}


def _build_program():
    import concourse.bass as bass
    import concourse.bacc as bacc
    import concourse.mybir as mybir
    import concourse.tile as tile
    from contextlib import ExitStack

    f32 = mybir.dt.float32
    f32r = mybir.dt.float32r
    bf16 = mybir.dt.bfloat16
    AX = mybir.AxisListType
    OP = mybir.AluOpType
    ACT = mybir.ActivationFunctionType

    nc = bacc.Bacc("TRN2", target_bir_lowering=False, debug=False)

    x = nc.dram_tensor("x", [B, C, N], f32, kind="ExternalInput").ap()
    g128 = nc.dram_tensor("gamma128", [128, 1], f32, kind="ExternalInput").ap()
    ident_d = nc.dram_tensor("ident", [128, 128], f32, kind="ExternalInput").ap()
    y = nc.dram_tensor("y", [B, C, N], f32, kind="ExternalOutput").ap()

    with ExitStack() as ctx:
        tc = ctx.enter_context(tile.TileContext(nc))
        const_p = ctx.enter_context(tc.tile_pool(name="const", bufs=1))
        # qnat (fp32 q) and qT (f32r transposed q) alternate through 2 slots;
        # batch b+1's qnat lands in the slot freed by batch b's qT so its DMA
        # overlaps batch b's value-matmul phase.
        big_p = ctx.enter_context(tc.tile_pool(name="big", bufs=2))
        q_p = ctx.enter_context(tc.tile_pool(name="qq", bufs=1))
        qbf_p = ctx.enter_context(tc.tile_pool(name="qbf", bufs=1))
        tmp_p = ctx.enter_context(tc.tile_pool(name="tmp", bufs=2))
        sm_p = ctx.enter_context(tc.tile_pool(name="sm", bufs=2))
        rep_p = ctx.enter_context(tc.tile_pool(name="rep", bufs=1))
        osb_p = ctx.enter_context(tc.tile_pool(name="osb", bufs=8))
        ps = ctx.enter_context(tc.tile_pool(name="ps", bufs=8, space="PSUM"))

        ident = const_p.tile([128, 128], f32, tag="ident")
        nc.sync.dma_start(ident[:], ident_d)
        gam = const_p.tile([128, 1], f32, tag="gam")
        nc.sync.dma_start(gam[:], g128)
        ones128 = const_p.tile([128, 1], bf16, tag="ones128")
        nc.gpsimd.memset(ones128[:], 1.0)
        ones1 = const_p.tile([1, 128], f32, tag="ones1")
        nc.gpsimd.memset(ones1[:], 1.0)

        # warm the PE clock during the initial DMA wait: dummy transposes of
        # the identity keep the ramp/HAM window busy so the first real
        # transposes run at full clock
        warm = ps.tile([128, 512], f32, tag="ps", name="warm")
        for w in range(8):
            nc.tensor.matmul(
                warm[:, 128 * (w % 4):128 * (w % 4 + 1)],
                ident[:],
                ident[:],
                is_transpose=True,
                skip_group_check=True,
            )

        for b in range(B):
            # ---- load q in natural layout, chunked so transposes can
            #      start as soon as the first columns land
            qnat = big_p.tile([128, CT, N], f32, tag="big")
            for t in range(CT):
                for lo, hi in [(0, 128), (128, 512)]:
                    nc.sync.dma_start(
                        qnat[:, t, lo:hi],
                        x[b, 128 * t:128 * (t + 1), lo:hi],
                    )
                for h in range(1, 8):
                    nc.sync.dma_start(
                        qnat[:, t, 512 * h:512 * (h + 1)],
                        x[b, 128 * t:128 * (t + 1), 512 * h:512 * (h + 1)],
                    )

            # ---- build qT[n_part, c_free]; 4 transposes per PSUM bank, one
            #      [128,512] DVE copy per bank (rounds to f32r)
            qt = big_p.tile([128, NT, C], f32r, tag="big")
            for t in range(CT):
                for jq in range(NT // 4):
                    tp4 = ps.tile([128, 512], f32, tag="ps")
                    for i in range(4):
                        j = 4 * jq + i
                        nc.tensor.matmul(
                            tp4[:, 128 * i:128 * (i + 1)],
                            qnat[:, t, 128 * j:128 * (j + 1)],
                            ident[:],
                            is_transpose=True,
                            skip_group_check=True,
                        )
                    nc.vector.tensor_copy(
                        qt[:, 4 * jq:4 * (jq + 1), 128 * t:128 * (t + 1)],
                        tp4[:].rearrange("p (a c) -> p a c", a=4),
                    )

            # ---- bf16 copy of q for the value matmul, on the idle scalar
            #      engine (ACT)
            qbf = qbf_p.tile([128, CT, N], bf16, tag="qbf")
            for t in range(CT):
                nc.scalar.copy(qbf[:, t, :], qnat[:, t, :])

            # ---- energy: E is symmetric, so compute only columns
            #      [lo_t:512] per row-tile (lo capped at 256: narrower f32r
            #      moving operands drop to 1/4 rate) and mirror the missing
            #      [128,128] blocks by transposing the stored ones.
            elo = [0, 128, 256, 256]
            mirrors = {0: [(0, 1), (0, 2), (0, 3)], 1: [(1, 2), (1, 3)]}
            rmins = sm_p.tile([128, CT], f32, tag="rmins")
            colrep_ps = ps.tile([128, C], f32, tag="ps")
            E = [ps.tile([128, C], f32, tag="ps", name=f"Et{t_}")
                 for t_ in range(CT)]
            for t in range(CT):
                Et = E[t]
                for j in range(NT):
                    nc.tensor.matmul(
                        Et[:, elo[t]:C],
                        qt[:, j, 128 * t:128 * (t + 1)],
                        qt[:, j, elo[t]:C],
                        start=(j == 0),
                        stop=(j == NT - 1),
                    )
                # mirror blocks sourced from tile t into later tiles' banks
                # (target regions are disjoint from their MM-written ranges,
                # so this can precede those tiles' accumulation)
                for s, tt in mirrors.get(t, []):
                    blk = sm_p.tile([128, 128], f32, tag="mirror")
                    nc.vector.tensor_copy(
                        blk[:], E[s][:, 128 * tt:128 * (tt + 1)]
                    )
                    nc.tensor.matmul(
                        E[tt][:, 128 * s:128 * (s + 1)],
                        blk[:],
                        ident[:],
                        is_transpose=True,
                        skip_group_check=True,
                    )
                # tile t of E is now complete (its own MMs + any mirrors
                # emitted in earlier iterations): fold its stats immediately
                # so only tile 3's chain trails the energy phase
                nc.vector.tensor_reduce(
                    rmins[:, t:t + 1], E[t][:], axis=AX.X, op=OP.min
                )
                tpm = ps.tile([1, 128], f32, tag="ps")
                nc.tensor.transpose(tpm[:], rmins[:, t:t + 1], ident[:])
                stT = sm_p.tile([1, 128], f32, tag="stT")
                nc.vector.tensor_copy(stT[:], tpm[:])
                nc.tensor.matmul(
                    colrep_ps[:, 128 * t:128 * (t + 1)],
                    ones1[:],
                    stT[:],
                    start=True,
                    stop=True,
                )
            colrep = rep_p.tile([128, C], f32, tag="colrep")
            nc.vector.tensor_copy(colrep[:], colrep_ps[:])

            # ---- U[d, c] = exp(min_c - E[d, c])  (<= 1, no overflow)
            U = q_p.tile([128, CT, C], bf16, tag="qq")
            for t in range(CT):
                tmp = tmp_p.tile([128, C], f32, tag="tmp")
                nc.vector.tensor_tensor(
                    tmp[:], colrep[:], E[t][:], op=OP.subtract
                )
                nc.scalar.activation(U[:, t, :], tmp[:], ACT.Exp)

            # ---- out[c, n] = scale_c * sum_d U[d, c] q[d, n] + x[c, n]
            #      R_c = sum_d U[d, c] (PE ones-reduction) is interleaved
            #      per m so the first value matmuls start sooner;
            #      scale_m = gamma / max(R, tiny) per output partition
            for m in range(CT):
                Rp = ps.tile([128, 1], f32, tag="ps")
                for k in range(CT):
                    nc.tensor.matmul(
                        Rp[:],
                        U[:, k, 128 * m:128 * (m + 1)],
                        ones128[:],
                        start=(k == 0),
                        stop=(k == CT - 1),
                    )
                Rsb = sm_p.tile([128, 1], f32, tag="rsb")
                nc.vector.tensor_scalar_max(Rsb[:], Rp[:], 1e-38)
                rec = sm_p.tile([128, 1], f32, tag="rec")
                nc.vector.reciprocal(rec[:], Rsb[:])
                sc = sm_p.tile([128, 1], f32, tag=f"scale{m}")
                nc.vector.tensor_scalar_mul(sc[:], rec[:], gam[:, 0:1])
                O = []
                for n in range(NCH):
                    On = ps.tile([128, 512], f32, tag="ps")
                    O.append(On)
                for k in range(CT):
                    for n in range(NCH):
                        nc.tensor.matmul(
                            O[n][:],
                            U[:, k, 128 * m:128 * (m + 1)],
                            qbf[:, k, 512 * n:512 * (n + 1)],
                            start=(k == 0),
                            stop=(k == CT - 1),
                            skip_group_check=True,
                        )
                for n in range(NCH):
                    osb = osb_p.tile([128, 512], f32, tag="osb")
                    nc.vector.scalar_tensor_tensor(
                        osb[:],
                        O[n][:],
                        sc[:],
                        qnat[:, m, 512 * n:512 * (n + 1)],
                        op0=OP.mult,
                        op1=OP.add,
                    )
                    nc.sync.dma_start(
                        y[b, 128 * m:128 * (m + 1), 512 * n:512 * (n + 1)],
                        osb[:],
                    )

    nc.compile()
    return nc


def get_program():
    if "nc" not in _cache:
        _cache["nc"] = _build_program()
    return _cache["nc"]


def kernel(x, gamma):
    from concourse.bass_utils import run_bass_kernel_spmd

    nc = get_program()
    xr = np.ascontiguousarray(
        np.asarray(x, dtype=np.float32).reshape(B_TOTAL, C, N)
    )
    g = np.asarray(gamma, dtype=np.float32).reshape(1)
    g128 = np.ascontiguousarray(
        np.broadcast_to(g.reshape(1, 1), (128, 1))
    ).astype(np.float32)
    ident = np.eye(128, dtype=np.float32)
    in_maps = [
        {
            "x": xr[i * B:(i + 1) * B],
            "gamma128": g128,
            "ident": ident,
        }
        for i in range(NCORES)
    ]
    res = run_bass_kernel_spmd(nc, in_maps, list(range(NCORES))).results
    y = np.concatenate([res[i]["y"] for i in range(NCORES)], axis=0)
    return y.reshape(B_TOTAL, C, H, W).astype(np.float32)



# revision 47
# speedup vs baseline: 1.4129x; 1.4129x over previous
"""Trainium2 Bass kernel for the CAM (channel attention) module.

Computes, per batch element b:
    q = x[b].reshape(C, N)                      # C=512, N=4096
    E = q @ q.T                                 # C x C  (symmetric)
    att = softmax(rowmax(E) - E, axis=-1)       # == softmax(-E) row-wise
    out = gamma * (att @ q) + x[b]

Sharding: data-parallel over batch. 16 batch elements -> 2 per NeuronCore
across 8 cores. gamma replicated. No collectives.

The kernel is DMA-bound: 16.8 MB in + 16.8 MB out per core at the 360 GB/s
aggregate DMA roofline is ~93 us. All engine budgets are kept below that and
the schedule keeps the DMA stream gapless:

  1. All input DMAs ([128,1024] pieces, both batches) are issued up front on
     SP so no input transfer ever queues behind an output DMA's wait.
  2. q is transposed on PE into a 4-slot SBUF ring (one [128,C] f32r tile per
     128-column chunk j), drained from PSUM by the scalar engine (rounding to
     f32r, which the energy matmul needs for full PE rate). The small ring
     (vs. a full [N,C] transpose buffer) frees SBUF to double-buffer the
     natural-layout q, so batch b+1's loads/compute never wait on batch b's
     residual phase.
  3. E tiles accumulate in 4 PSUM banks over the j-stream; symmetry halves
     the matmul work (elo trick) and 5 mirrored [128,128] blocks are filled
     by PE transposes at the end.
  4. Column-oriented softmax as in the max-trick: att_T[d,c]=exp(min_c -
     E[d,c])/R_c. exp argument <= 0, no overflow; R >= 1 so the reciprocal
     is safe (clamped anyway). U is written in fp8e4.
  5. The value matmul att @ q runs in fp8e4 DoubleRow perf mode (2 k-tiles
     per pass) against a fp8 copy of q produced on the (otherwise idle)
     gpsimd engine. gamma/R_c and the +x residual are fused into one DVE
     scalar_tensor_tensor per 512-column chunk; x enters only here, in exact
     fp32, so for gamma == 0 the kernel output is bit-exact x.
"""

import sys

import numpy as np

_REPO = "/opt/trn_rl_repo"
if _REPO not in sys.path:
    sys.path.insert(0, _REPO)

B_TOTAL, C, H, W = 16, 512, 64, 64
N = H * W          # 4096
NCORES = 8
B = B_TOTAL // NCORES  # batches per core = 2
CT = C // 128      # 4 c-tiles
NT = N // 128      # 32 n-tiles
NCH = N // 512     # 8 output column chunks
NG = N // 1024     # 4 input DMA column groups

_cache = {}


def _build_program():
    import concourse.bass as bass
    import concourse.bacc as bacc
    import concourse.mybir as mybir
    import concourse.tile as tile
    from contextlib import ExitStack

    f32 = mybir.dt.float32
    f32r = mybir.dt.float32r
    fp8 = mybir.dt.float8e4
    AX = mybir.AxisListType
    OP = mybir.AluOpType
    ACT = mybir.ActivationFunctionType
    DR = mybir.MatmulPerfMode.DoubleRow

    nc = bacc.Bacc("TRN2", target_bir_lowering=False, debug=False)

    # x is declared f32r so the natural-layout q tiles can feed the PE
    # transposes at the f32r rate (1.5 cycles/row vs fp32's 2). f32r is
    # bit-identical fp32 in memory; the DMA just moves bytes.
    x = nc.dram_tensor("x", [B, C, N], f32r, kind="ExternalInput").ap()
    g128 = nc.dram_tensor("gamma128", [128, 1], f32, kind="ExternalInput").ap()
    ident_d = nc.dram_tensor("ident", [128, 128], f32, kind="ExternalInput").ap()
    identr_d = nc.dram_tensor(
        "identr", [128, 128], f32r, kind="ExternalInput"
    ).ap()
    y = nc.dram_tensor("y", [B, C, N], f32, kind="ExternalOutput").ap()

    with ExitStack() as ctx:
        tc = ctx.enter_context(tile.TileContext(nc))
        const_p = ctx.enter_context(tc.tile_pool(name="const", bufs=1))
        qnat_p = ctx.enter_context(tc.tile_pool(name="qnat", bufs=2))
        qt_p = ctx.enter_context(tc.tile_pool(name="qt", bufs=7))
        q8_p = ctx.enter_context(tc.tile_pool(name="q8", bufs=2))
        u_p = ctx.enter_context(tc.tile_pool(name="u", bufs=2))
        unat_p = ctx.enter_context(tc.tile_pool(name="unat", bufs=4))
        sm_p = ctx.enter_context(tc.tile_pool(name="sm", bufs=2))
        osb_p = ctx.enter_context(tc.tile_pool(name="osb", bufs=4))
        # PSUM: tags are separate rings. E x4 + tp x2 + O x2 = 8 banks.
        ps = ctx.enter_context(tc.tile_pool(name="ps", bufs=1, space="PSUM"))

        ident = const_p.tile([128, 128], f32, tag="ident")
        nc.sync.dma_start(ident[:], ident_d)
        identr = const_p.tile([128, 128], f32r, tag="identr")
        nc.sync.dma_start(identr[:], identr_d)
        gam = const_p.tile([128, 1], f32, tag="gam")
        nc.sync.dma_start(gam[:], g128)
        qnat0 = qnat_p.tile([128, CT, N], f32r, tag="qnat")
        nc.sync.dma_start(qnat0[:, 0, 0:512], x[0, 0:128, 0:512])

        def tp_tile(shape, name, dtype=f32):
            return ps.tile(shape, dtype, tag="tp", name=name, bufs=2)

        def o_tile(name):
            return ps.tile([128, 512], f32, tag="O", name=name, bufs=2)

        # warm the PE clock during the initial DMA wait: a long dependency-
        # free transpose chain keeps PE continuously busy through the p-state
        # ramp so the first real transposes run at full clock.
        warm = tp_tile([128, 512], "warm", f32r)
        for w in range(16):
            nc.tensor.matmul(
                warm[:, 128 * (w % 4):128 * (w % 4 + 1)],
                identr[:],
                identr[:],
                is_transpose=True,
                skip_group_check=True,
            )

        # ---- all input DMAs up front (SP queue: nothing ahead of them can
        #      block), plus the fp8 cast of q on the gpsimd engine, piecewise
        #      so it tracks the DMA stream. Batch 0's first column group is
        #      split finer so the transpose pipeline starts sooner.
        qnats, q8s = [qnat0], []
        for b in range(B):
            qnat = qnats[b] if b == 0 else qnat_p.tile(
                [128, CT, N], f32r, tag="qnat", name="qnat"
            )
            q8 = q8_p.tile([128, CT, N], fp8, tag="q8")
            for g in range(NG):
                if b == 0 and g == 0:
                    for h in range(2):
                        for t in range(CT):
                            if h == 0 and t == 0:
                                continue  # issued before the consts
                            nc.sync.dma_start(
                                qnat[:, t, 512 * h:512 * (h + 1)],
                                x[b, 128 * t:128 * (t + 1),
                                  512 * h:512 * (h + 1)],
                            )
                else:
                    for t in range(CT):
                        nc.sync.dma_start(
                            qnat[:, t, 1024 * g:1024 * (g + 1)],
                            x[b, 128 * t:128 * (t + 1),
                              1024 * g:1024 * (g + 1)],
                        )
                for t in range(CT):
                    nc.gpsimd.tensor_copy(
                        q8[:, t, 1024 * g:1024 * (g + 1)],
                        qnat[:, t, 1024 * g:1024 * (g + 1)],
                    )
            if b > 0:
                qnats.append(qnat)
            q8s.append(q8)

        elo = [0, 128, 256, 256]
        mirrors = [(0, 1), (0, 2), (1, 2), (0, 3), (1, 3)]
        Es = [None] * B
        Us = [None] * B
        scs = [[None] * CT for _ in range(B)]

        qts = [[None] * NT for _ in range(B)]

        def emit_tp(b, j):
            """Transpose chunk j (4 row-tiles) into one PSUM bank and drain
            to the qt ring (scalar engine, rounds to f32r)."""
            qnat = qnats[b]
            if j == 0:
                Es[b] = [
                    ps.tile([128, C], f32, tag="E", name=f"E{b}t{t_}", bufs=4)
                    for t_ in range(CT)
                ]
            tp = tp_tile([128, 512], f"tp{b}j{j}", f32r)
            for t in range(CT):
                nc.tensor.matmul(
                    tp[:, 128 * t:128 * (t + 1)],
                    qnat[:, t, 128 * j:128 * (j + 1)],
                    identr[:],
                    is_transpose=True,
                    skip_group_check=True,
                )
            qt = qt_p.tile([128, C], f32r, tag="qt")
            nc.scalar.copy(qt[:], tp[:])
            qts[b][j] = qt

        def emit_E(b, j):
            """Accumulate the 4 E tiles from qt chunk j."""
            E, qt = Es[b], qts[b][j]
            for t in range(CT):
                nc.tensor.matmul(
                    E[t][:, elo[t]:C],
                    qt[:, 128 * t:128 * (t + 1)],
                    qt[:, elo[t]:C],
                    start=(j == 0),
                    stop=(j == NT - 1),
                )

        def emit_post_energy(b, value_per_tile=False, filler=None):
            """Softmax via the symmetric-bias trick, per row-tile t:

            U_nat[t][c', d] = exp(rowmin_c - E[c, d]) with c = 128t + c' is
            exactly the att numerator transposed per c-tile (E symmetric, so
            the column-min of column c equals the row-min of row c, which is
            a per-PARTITION bias for the tile). The activation's accumulator
            gives R_c = sum_d U for free, and E[t]'s PSUM bank is freed right
            at the exp — unblocking the next batch's energy accumulation.
            PE then transposes U_nat into the [d, c] orientation the value
            matmul needs (16 cheap f32r transposes + 1 drain per tile)."""
            E = Es[b]
            rmins = sm_p.tile([128, CT], f32, tag="rmins")
            # tile 0 has no incoming mirror blocks: start its row-min before
            # the mirror copies so its exp (which frees E[0]'s bank) fires
            # as early as possible
            nc.vector.tensor_reduce(
                rmins[:, 0:1], E[0][:], axis=AX.X, op=OP.min
            )
            for s, tt in mirrors:
                blk = sm_p.tile(
                    [128, 128], f32, tag="mirror", name=f"blk{b}{s}{tt}",
                    bufs=3,
                )
                nc.vector.tensor_copy(blk[:], E[s][:, 128 * tt:128 * (tt + 1)])
                nc.tensor.matmul(
                    E[tt][:, 128 * s:128 * (s + 1)],
                    blk[:],
                    ident[:],
                    is_transpose=True,
                    skip_group_check=True,
                )
            # remaining row-mins up front: they are the only gate on the ACT
            # exp chain, which in turn frees the E banks for the next batch
            for t in range(1, CT):
                nc.vector.tensor_reduce(
                    rmins[:, t:t + 1], E[t][:], axis=AX.X, op=OP.min
                )
            U = u_p.tile([128, CT, C], fp8, tag="U")
            for t in range(CT):
                unat = unat_p.tile([128, C], f32r, tag="unat")
                R = sm_p.tile([128, 1], f32, tag="R")
                nc.scalar.activation(
                    unat[:], E[t][:], ACT.Exp,
                    bias=rmins[:, t:t + 1], scale=-1.0, accum_out=R[:],
                )
                # R >= 1 always: the row's own min term contributes exp(0)
                rec = sm_p.tile([128, 1], f32, tag="rec")
                nc.vector.reciprocal(rec[:], R[:])
                sc = sm_p.tile([128, 1], f32, tag=f"scale{t}")
                nc.vector.tensor_scalar_mul(sc[:], rec[:], gam[:, 0:1])
                scs[b][t] = sc
                utp = tp_tile([128, 512], f"utp{b}{t}", f32r)
                for k in range(CT):
                    nc.tensor.matmul(
                        utp[:, 128 * k:128 * (k + 1)],
                        unat[:, 128 * k:128 * (k + 1)],
                        identr[:],
                        is_transpose=True,
                        skip_group_check=True,
                    )
                nc.scalar.copy(
                    U[:, 0:CT, 128 * t:128 * (t + 1)],
                    utp[:].rearrange("p (k c) -> p k c", k=CT),
                )
                Us[b] = U
                if value_per_tile:
                    # first pair of each tile takes the short all-DVE path;
                    # later pairs use the split path for throughput
                    for nh in range(NCH // 2):
                        idx = t * (NCH // 2) + nh
                        emit_value_pair(
                            b, idx, split=(nh > 0),
                            tail=(idx == 0 or idx >= NT // 2 - 2),
                        )
                if filler is not None:
                    filler()

        def emit_value_pair(b, idx, split=False, tail=False):
            """Two 512-column value chunks (fp8 DoubleRow) + fused residual
            + one [128,1024] output DMA. idx in [0, 16).

            split=False: both residuals as DVE STTs (PSUM direct).
            split=True: chunk 1's residual goes ACT (scale-copy from PSUM)
            then gpsimd (in-place +x), doubling pair production rate when
            DVE would otherwise pace the tail."""
            m, nh = idx // (NCH // 2), idx % (NCH // 2)
            U, q8, qnat, sc = Us[b], q8s[b], qnats[b], scs[b][m]
            osb = osb_p.tile([128, 1024], f32, tag="osb")
            for i in range(2):
                n = 2 * nh + i
                On = o_tile(f"O{b}{m}{n}")
                for p in range(CT // 2):
                    nc.tensor.matmul(
                        On[:],
                        U[:, 2 * p:2 * p + 2, 128 * m:128 * (m + 1)],
                        q8[:, 2 * p:2 * p + 2, 512 * n:512 * (n + 1)],
                        start=(p == 0),
                        stop=(p == CT // 2 - 1),
                        perf_mode=DR,
                        skip_group_check=True,
                    )
                half = osb[:, 512 * i:512 * (i + 1)]
                xs = qnat[:, m, 512 * n:512 * (n + 1)]
                if split and i == 1:
                    nc.scalar.activation(half, On[:], ACT.Copy, scale=sc[:])
                    nc.gpsimd.tensor_tensor(half, half, xs, op=OP.add)
                else:
                    nc.vector.scalar_tensor_tensor(
                        half, On[:], sc[:], xs, op0=OP.mult, op1=OP.add
                    )
                if tail:
                    # last pairs ship per-chunk so the final transfer isn't
                    # held for the second chunk's residual
                    nc.sync.dma_start(
                        y[b, 128 * m:128 * (m + 1),
                          512 * n:512 * (n + 1)],
                        half,
                    )
            if not tail:
                nc.sync.dma_start(
                    y[b, 128 * m:128 * (m + 1), 1024 * nh:1024 * (nh + 1)],
                    osb[:],
                )

        # ---- software-pipelined schedule. Two pipelining levels:
        #      * within a batch's energy stream, chunk j+1's transposes are
        #        emitted before chunk j's E matmuls so the in-order PE queue
        #        never stalls on the PSUM->SBUF drain round-trip;
        #      * batch 1's energy stream is interleaved with batch 0's value
        #        phase so the PE never sits behind the (STT-paced) value
        #        chunks.
        # Lookahead-2 on both energy streams hides the PSUM->SBUF drain
        # round-trip (with lookahead-1 the E group arrives ~160ns before the
        # drain ack). Batch 0's value pairs are spaced 2-per-3 energy chunks:
        # the 2-deep O-ring ties each pair's matmuls to the previous pair's
        # (DVE) STT, so tighter spacing would stall the in-order PE queue.
        LA = 2
        for j in range(LA):
            emit_tp(0, j)
        for j in range(LA, NT + LA):
            if j < NT:
                emit_tp(0, j)
            emit_E(0, j - LA)
        # batch 1's transpose stream is threaded through batch 0's post-
        # energy as PE filler: PE would otherwise idle behind the serialized
        # ACT exp chain there.
        k_tp = 0

        def tp_filler():
            nonlocal k_tp
            for _ in range(3):
                if k_tp < NT:
                    emit_tp(1, k_tp)
                    k_tp += 1

        emit_post_energy(0, filler=tp_filler)
        k_E = 0
        next_pair = 0
        while k_E < NT:
            if k_tp < NT:
                emit_tp(1, k_tp)
                k_tp += 1
            if k_tp - k_E >= LA or k_tp >= NT:
                emit_E(1, k_E)
                k_E += 1
                if k_E % 4 != 0 and next_pair < NT // 2:
                    emit_value_pair(0, next_pair)
                    next_pair += 1
        while next_pair < NT // 2:
            emit_value_pair(0, next_pair)
            next_pair += 1
        # batch 1's post-energy pipelines per c-tile: each tile's U slice and
        # scale feed its 4 output pairs immediately, so the output stream
        # starts ~3us after the last E matmul instead of after the full
        # softmax pass.
        emit_post_energy(1, value_per_tile=True)

    nc.compile()
    return nc


def get_program():
    if "nc" not in _cache:
        _cache["nc"] = _build_program()
    return _cache["nc"]


def kernel(x, gamma):
    from concourse.bass_utils import run_bass_kernel_spmd

    nc = get_program()
    xr = np.ascontiguousarray(
        np.asarray(x, dtype=np.float32).reshape(B_TOTAL, C, N)
    )
    g = np.asarray(gamma, dtype=np.float32).reshape(1)
    g128 = np.ascontiguousarray(
        np.broadcast_to(g.reshape(1, 1), (128, 1))
    ).astype(np.float32)
    ident = np.eye(128, dtype=np.float32)
    in_maps = [
        {
            "x": xr[i * B:(i + 1) * B],
            "gamma128": g128,
            "ident": ident,
            "identr": ident,
        }
        for i in range(NCORES)
    ]
    res = run_bass_kernel_spmd(nc, in_maps, list(range(NCORES))).results
    y = np.concatenate([res[i]["y"] for i in range(NCORES)], axis=0)
    return y.reshape(B_TOTAL, C, H, W).astype(np.float32)


# revision 62
# speedup vs baseline: 1.4315x; 1.0131x over previous
"""Trainium2 Bass kernel for the CAM (channel attention) module.

Computes, per batch element b:
    q = x[b].reshape(C, N)                      # C=512, N=4096
    E = q @ q.T                                 # C x C  (symmetric)
    att = softmax(rowmax(E) - E, axis=-1)       # == softmax(-E) row-wise
    out = gamma * (att @ q) + x[b]

Sharding: data-parallel over batch. 16 batch elements -> 2 per NeuronCore
across 8 cores. gamma replicated. No collectives.

The kernel is DMA-bound: 16.8 MB in + 16.8 MB out per core at the 360 GB/s
aggregate DMA roofline is ~93 us. All engine budgets are kept below that and
the emission order software-pipelines everything around the DMA stream:

  1. All input DMAs (both batches, [128,1024] pieces) are issued up front on
     SP so no input transfer ever queues behind an output DMA's wait.
  2. q is transposed on PE (f32r datapath, 1.5 cycles/row) into a 7-slot
     SBUF ring, one [128,C] f32r tile per 128-column chunk j, drained from a
     2-bank PSUM ring by the scalar engine. The small ring (vs. a full [N,C]
     transpose buffer) frees SBUF to double-buffer the natural-layout q, so
     batch 1's loads and compute never wait on batch 0's residual phase.
  3. E tiles accumulate in 4 PSUM banks over the j-stream with a 2-chunk
     lookahead that hides the drain round-trip; symmetry saves 25% of the
     matmul work (elo trick) and 5 mirrored [128,128] blocks are filled by
     PE transposes at the end.
  4. Softmax via the symmetric-bias trick (see emit_post_energy): the exp
     runs directly on the PSUM E tile with the row-min as per-partition
     bias, its accumulator yields R for free, and the E bank is released at
     the exp itself — the next batch's energy stream is gated on exactly
     this. exp argument <= 0 (no overflow), R >= 1 (safe reciprocal).
  5. The value matmul att @ q runs in fp8e4 DoubleRow perf mode (2 k-tiles
     per pass, 0.5 cycles/row) against a fp8 copy of q produced on the
     otherwise-idle gpsimd engine during the load phase. gamma/R_c and the
     +x residual are fused into one DVE scalar_tensor_tensor per 512-column
     chunk (with some chunks routed ACT scale-copy + gpsimd add in the final
     tail to double production rate); x enters only here, so for gamma == 0
     the attention path cannot perturb the output beyond f32r rounding.
  6. Batch 1's energy stream is interleaved (via emission order) with batch
     0's value phase, and batch 1's softmax tiles pipeline straight into
     their output pairs, so the output DMA tail starts as early as the PE
     work total allows.
"""

import sys

import numpy as np

_REPO = "/opt/trn_rl_repo"
if _REPO not in sys.path:
    sys.path.insert(0, _REPO)

B_TOTAL, C, H, W = 16, 512, 64, 64
N = H * W          # 4096
NCORES = 8
B = B_TOTAL // NCORES  # batches per core = 2
CT = C // 128      # 4 c-tiles
NT = N // 128      # 32 n-tiles
NCH = N // 512     # 8 output column chunks
NG = N // 1024     # 4 input DMA column groups

_cache = {}


def _build_program():
    import concourse.bass as bass
    import concourse.bacc as bacc
    import concourse.mybir as mybir
    import concourse.tile as tile
    from contextlib import ExitStack

    f32 = mybir.dt.float32
    f32r = mybir.dt.float32r
    fp8 = mybir.dt.float8e4
    AX = mybir.AxisListType
    OP = mybir.AluOpType
    ACT = mybir.ActivationFunctionType
    DR = mybir.MatmulPerfMode.DoubleRow

    nc = bacc.Bacc("TRN2", target_bir_lowering=False, debug=False)

    # x is declared f32r so the natural-layout q tiles can feed the PE
    # transposes at the f32r rate (1.5 cycles/row vs fp32's 2). f32r is
    # bit-identical fp32 in memory; the DMA just moves bytes.
    x = nc.dram_tensor("x", [B, C, N], f32r, kind="ExternalInput").ap()
    g128 = nc.dram_tensor("gamma128", [128, 1], f32, kind="ExternalInput").ap()
    ident_d = nc.dram_tensor("ident", [128, 128], f32, kind="ExternalInput").ap()
    identr_d = nc.dram_tensor(
        "identr", [128, 128], f32r, kind="ExternalInput"
    ).ap()
    y = nc.dram_tensor("y", [B, C, N], f32, kind="ExternalOutput").ap()

    with ExitStack() as ctx:
        tc = ctx.enter_context(tile.TileContext(nc))
        const_p = ctx.enter_context(tc.tile_pool(name="const", bufs=1))
        qnat_p = ctx.enter_context(tc.tile_pool(name="qnat", bufs=2))
        qt_p = ctx.enter_context(tc.tile_pool(name="qt", bufs=7))
        q8_p = ctx.enter_context(tc.tile_pool(name="q8", bufs=2))
        u_p = ctx.enter_context(tc.tile_pool(name="u", bufs=2))
        unat_p = ctx.enter_context(tc.tile_pool(name="unat", bufs=4))
        sm_p = ctx.enter_context(tc.tile_pool(name="sm", bufs=2))
        osb_p = ctx.enter_context(tc.tile_pool(name="osb", bufs=4))
        # PSUM: tags are separate rings. E x4 + tp x2 + O x2 = 8 banks.
        ps = ctx.enter_context(tc.tile_pool(name="ps", bufs=1, space="PSUM"))

        ident = const_p.tile([128, 128], f32, tag="ident")
        nc.sync.dma_start(ident[:], ident_d)
        identr = const_p.tile([128, 128], f32r, tag="identr")
        nc.sync.dma_start(identr[:], identr_d)
        gam = const_p.tile([128, 1], f32, tag="gam")
        nc.sync.dma_start(gam[:], g128)
        qnat0 = qnat_p.tile([128, CT, N], f32r, tag="qnat")
        nc.sync.dma_start(qnat0[:, 0, 0:512], x[0, 0:128, 0:512])

        def tp_tile(shape, name, dtype=f32):
            return ps.tile(shape, dtype, tag="tp", name=name, bufs=2)

        def o_tile(name):
            return ps.tile([128, 512], f32, tag="O", name=name, bufs=2)

        # warm the PE clock during the initial DMA wait: a long dependency-
        # free transpose chain keeps PE continuously busy through the p-state
        # ramp so the first real transposes run at full clock.
        warm = tp_tile([128, 512], "warm", f32r)
        for w in range(16):
            nc.tensor.matmul(
                warm[:, 128 * (w % 4):128 * (w % 4 + 1)],
                identr[:],
                identr[:],
                is_transpose=True,
                skip_group_check=True,
            )

        # ---- all input DMAs up front (SP queue: nothing ahead of them can
        #      block), plus the fp8 cast of q on the gpsimd engine, piecewise
        #      so it tracks the DMA stream. Batch 0's first column group is
        #      split finer so the transpose pipeline starts sooner.
        qnats, q8s = [qnat0], []
        for b in range(B):
            qnat = qnats[b] if b == 0 else qnat_p.tile(
                [128, CT, N], f32r, tag="qnat", name="qnat"
            )
            q8 = q8_p.tile([128, CT, N], fp8, tag="q8")
            for g in range(NG):
                if b == 0 and g == 0:
                    for h in range(2):
                        for t in range(CT):
                            if h == 0 and t == 0:
                                continue  # issued before the consts
                            nc.sync.dma_start(
                                qnat[:, t, 512 * h:512 * (h + 1)],
                                x[b, 128 * t:128 * (t + 1),
                                  512 * h:512 * (h + 1)],
                            )
                else:
                    for t in range(CT):
                        nc.sync.dma_start(
                            qnat[:, t, 1024 * g:1024 * (g + 1)],
                            x[b, 128 * t:128 * (t + 1),
                              1024 * g:1024 * (g + 1)],
                        )
                for t in range(CT):
                    nc.gpsimd.tensor_copy(
                        q8[:, t, 1024 * g:1024 * (g + 1)],
                        qnat[:, t, 1024 * g:1024 * (g + 1)],
                    )
            if b > 0:
                qnats.append(qnat)
            q8s.append(q8)

        elo = [0, 128, 256, 256]
        mirrors = [(0, 1), (0, 2), (1, 2), (0, 3), (1, 3)]
        Es = [None] * B
        Us = [None] * B
        scs = [[None] * CT for _ in range(B)]

        qts = [[None] * NT for _ in range(B)]

        def emit_tp(b, j):
            """Transpose chunk j (4 row-tiles) into one PSUM bank and drain
            to the qt ring (scalar engine, rounds to f32r)."""
            qnat = qnats[b]
            if j == 0:
                Es[b] = [
                    ps.tile([128, C], f32, tag="E", name=f"E{b}t{t_}", bufs=4)
                    for t_ in range(CT)
                ]
            tp = tp_tile([128, 512], f"tp{b}j{j}", f32r)
            for t in range(CT):
                nc.tensor.matmul(
                    tp[:, 128 * t:128 * (t + 1)],
                    qnat[:, t, 128 * j:128 * (j + 1)],
                    identr[:],
                    is_transpose=True,
                    skip_group_check=True,
                )
            qt = qt_p.tile([128, C], f32r, tag="qt")
            nc.scalar.copy(qt[:], tp[:])
            qts[b][j] = qt

        def emit_E(b, j):
            """Accumulate the 4 E tiles from qt chunk j."""
            E, qt = Es[b], qts[b][j]
            for t in range(CT):
                nc.tensor.matmul(
                    E[t][:, elo[t]:C],
                    qt[:, 128 * t:128 * (t + 1)],
                    qt[:, elo[t]:C],
                    start=(j == 0),
                    stop=(j == NT - 1),
                )

        def emit_post_energy(b, value_per_tile=False, filler=None):
            """Softmax via the symmetric-bias trick, per row-tile t:

            U_nat[t][c', d] = exp(rowmin_c - E[c, d]) with c = 128t + c' is
            exactly the att numerator transposed per c-tile (E symmetric, so
            the column-min of column c equals the row-min of row c, which is
            a per-PARTITION bias for the tile). The activation's accumulator
            gives R_c = sum_d U for free, and E[t]'s PSUM bank is freed right
            at the exp — unblocking the next batch's energy accumulation.
            PE then transposes U_nat into the [d, c] orientation the value
            matmul needs (16 cheap f32r transposes + 1 drain per tile)."""
            E = Es[b]
            rmins = sm_p.tile([128, CT], f32, tag="rmins")
            U = u_p.tile([128, CT, C], fp8, tag="U")
            Us[b] = U

            def tile_chain(t):
                unat = unat_p.tile(
                    [128, C], f32r, tag="unat", name=f"unat{b}{t}"
                )
                R = sm_p.tile([128, 1], f32, tag="R", name=f"R{b}{t}")
                nc.scalar.activation(
                    unat[:], E[t][:], ACT.Exp,
                    bias=rmins[:, t:t + 1], scale=-1.0, accum_out=R[:],
                )
                # R >= 1 always: the row's own min term contributes exp(0)
                rec = sm_p.tile([128, 1], f32, tag="rec", name=f"rec{b}{t}")
                nc.vector.reciprocal(rec[:], R[:])
                sc = sm_p.tile([128, 1], f32, tag=f"scale{t}")
                nc.vector.tensor_scalar_mul(sc[:], rec[:], gam[:, 0:1])
                scs[b][t] = sc
                utp = tp_tile([128, 512], f"utp{b}{t}", f32r)
                for k in range(CT):
                    nc.tensor.matmul(
                        utp[:, 128 * k:128 * (k + 1)],
                        unat[:, 128 * k:128 * (k + 1)],
                        identr[:],
                        is_transpose=True,
                        skip_group_check=True,
                    )
                nc.scalar.copy(
                    U[:, 0:CT, 128 * t:128 * (t + 1)],
                    utp[:].rearrange("p (k c) -> p k c", k=CT),
                )
                if value_per_tile == "first":
                    # prime one output pair per tile; the rest follow in the
                    # interleaved stream
                    emit_value_pair(b, t * (NCH // 2))
                elif value_per_tile:
                    # first pair of each tile takes the short all-DVE path;
                    # later pairs use the split path for throughput
                    for nh in range(NCH // 2):
                        idx = t * (NCH // 2) + nh
                        emit_value_pair(
                            b, idx, split=(nh > 0),
                            tail=(idx == 0 or idx >= NT // 2 - 2),
                        )
                if filler is not None:
                    filler()

            # tile 0 has no incoming mirror blocks: its whole chain (row-min
            # -> exp -> U slice -> first output pairs) runs ahead of the
            # mirror work, which only tiles 1-3 need
            nc.vector.tensor_reduce(
                rmins[:, 0:1], E[0][:], axis=AX.X, op=OP.min
            )
            tile_chain(0)
            for s, tt in mirrors:
                blk = sm_p.tile(
                    [128, 128], f32, tag="mirror", name=f"blk{b}{s}{tt}",
                    bufs=3,
                )
                nc.vector.tensor_copy(blk[:], E[s][:, 128 * tt:128 * (tt + 1)])
                nc.tensor.matmul(
                    E[tt][:, 128 * s:128 * (s + 1)],
                    blk[:],
                    ident[:],
                    is_transpose=True,
                    skip_group_check=True,
                )
            for t in range(1, CT):
                nc.vector.tensor_reduce(
                    rmins[:, t:t + 1], E[t][:], axis=AX.X, op=OP.min
                )
            for t in range(1, CT):
                tile_chain(t)

        def emit_value_pair(b, idx, split=False, tail=False):
            """Two 512-column value chunks (fp8 DoubleRow) + fused residual
            + one [128,1024] output DMA. idx in [0, 16).

            split=False: both residuals as DVE STTs (PSUM direct).
            split=True: chunk 1's residual goes ACT (scale-copy from PSUM)
            then gpsimd (in-place +x), doubling pair production rate when
            DVE would otherwise pace the tail."""
            m, nh = idx // (NCH // 2), idx % (NCH // 2)
            U, q8, qnat, sc = Us[b], q8s[b], qnats[b], scs[b][m]
            osb = osb_p.tile([128, 1024], f32, tag="osb")
            for i in range(2):
                n = 2 * nh + i
                On = o_tile(f"O{b}{m}{n}")
                for p in range(CT // 2):
                    nc.tensor.matmul(
                        On[:],
                        U[:, 2 * p:2 * p + 2, 128 * m:128 * (m + 1)],
                        q8[:, 2 * p:2 * p + 2, 512 * n:512 * (n + 1)],
                        start=(p == 0),
                        stop=(p == CT // 2 - 1),
                        perf_mode=DR,
                        skip_group_check=True,
                    )
                half = osb[:, 512 * i:512 * (i + 1)]
                xs = qnat[:, m, 512 * n:512 * (n + 1)]
                if split and i == 1:
                    nc.scalar.activation(half, On[:], ACT.Copy, scale=sc[:])
                    nc.gpsimd.tensor_tensor(half, half, xs, op=OP.add)
                else:
                    nc.vector.scalar_tensor_tensor(
                        half, On[:], sc[:], xs, op0=OP.mult, op1=OP.add
                    )
                if tail:
                    # last pairs ship per-chunk so the final transfer isn't
                    # held for the second chunk's residual
                    nc.sync.dma_start(
                        y[b, 128 * m:128 * (m + 1),
                          512 * n:512 * (n + 1)],
                        half,
                    )
            if not tail:
                nc.sync.dma_start(
                    y[b, 128 * m:128 * (m + 1), 1024 * nh:1024 * (nh + 1)],
                    osb[:],
                )

        # ---- software-pipelined schedule. Two pipelining levels:
        #      * within a batch's energy stream, chunk j+1's transposes are
        #        emitted before chunk j's E matmuls so the in-order PE queue
        #        never stalls on the PSUM->SBUF drain round-trip;
        #      * batch 1's energy stream is interleaved with batch 0's value
        #        phase so the PE never sits behind the (STT-paced) value
        #        chunks.
        # Lookahead-2 on both energy streams hides the PSUM->SBUF drain
        # round-trip (with lookahead-1 the E group arrives ~160ns before the
        # drain ack). Batch 0's value pairs are spaced 2-per-3 energy chunks:
        # the 2-deep O-ring ties each pair's matmuls to the previous pair's
        # (DVE) STT, so tighter spacing would stall the in-order PE queue.
        LA = 2
        for j in range(LA):
            emit_tp(0, j)
        for j in range(LA, NT + LA):
            if j < NT:
                emit_tp(0, j)
            emit_E(0, j - LA)
        # batch 1's transpose stream is threaded through batch 0's post-
        # energy as PE filler: PE would otherwise idle behind the serialized
        # ACT exp chain there.
        k_tp = 0

        def tp_filler():
            nonlocal k_tp
            for _ in range(3):
                if k_tp < NT:
                    emit_tp(1, k_tp)
                    k_tp += 1

        emit_post_energy(0, filler=tp_filler)
        k_E = 0
        next_pair = 0
        while k_E < NT:
            if k_tp < NT:
                emit_tp(1, k_tp)
                k_tp += 1
            if k_tp - k_E >= LA or k_tp >= NT:
                emit_E(1, k_E)
                k_E += 1
                if k_E % 4 != 0 and next_pair < NT // 2:
                    emit_value_pair(0, next_pair)
                    next_pair += 1
        while next_pair < NT // 2:
            emit_value_pair(0, next_pair)
            next_pair += 1
        # batch 1's post-energy pipelines per c-tile: each tile's U slice and
        # scale feed its 4 output pairs immediately, so the output stream
        # starts ~3us after the last E matmul instead of after the full
        # softmax pass.
        emit_post_energy(1, value_per_tile=True)

    nc.compile()
    return nc


def get_program():
    if "nc" not in _cache:
        _cache["nc"] = _build_program()
    return _cache["nc"]


def kernel(x, gamma):
    from concourse.bass_utils import run_bass_kernel_spmd

    nc = get_program()
    xr = np.ascontiguousarray(
        np.asarray(x, dtype=np.float32).reshape(B_TOTAL, C, N)
    )
    g = np.asarray(gamma, dtype=np.float32).reshape(1)
    g128 = np.ascontiguousarray(
        np.broadcast_to(g.reshape(1, 1), (128, 1))
    ).astype(np.float32)
    ident = np.eye(128, dtype=np.float32)
    in_maps = [
        {
            "x": xr[i * B:(i + 1) * B],
            "gamma128": g128,
            "ident": ident,
            "identr": ident,
        }
        for i in range(NCORES)
    ]
    res = run_bass_kernel_spmd(nc, in_maps, list(range(NCORES))).results
    y = np.concatenate([res[i]["y"] for i in range(NCORES)], axis=0)
    return y.reshape(B_TOTAL, C, H, W).astype(np.float32)


# revision 75
# speedup vs baseline: 1.4362x; 1.0033x over previous
"""Trainium2 Bass kernel for the CAM (channel attention) module.

Computes, per batch element b:
    q = x[b].reshape(C, N)                      # C=512, N=4096
    E = q @ q.T                                 # C x C  (symmetric)
    att = softmax(rowmax(E) - E, axis=-1)       # == softmax(-E) row-wise
    out = gamma * (att @ q) + x[b]

Sharding: data-parallel over batch. 16 batch elements -> 2 per NeuronCore
across 8 cores. gamma replicated. No collectives.

The kernel is DMA-bound: 16.8 MB in + 16.8 MB out per core at the 360 GB/s
aggregate DMA roofline is ~93 us. All engine budgets are kept below that and
the emission order software-pipelines everything around the DMA stream:

  1. All input DMAs (both batches, [128,1024] pieces) are issued up front on
     SP so no input transfer ever queues behind an output DMA's wait.
  2. q is transposed on PE (f32r datapath, 1.5 cycles/row) into a 7-slot
     SBUF ring, one [128,C] f32r tile per 128-column chunk j, drained from a
     2-bank PSUM ring by the scalar engine. The small ring (vs. a full [N,C]
     transpose buffer) frees SBUF to double-buffer the natural-layout q, so
     batch 1's loads and compute never wait on batch 0's residual phase.
  3. E tiles accumulate in 4 PSUM banks over the j-stream with a 2-chunk
     lookahead that hides the drain round-trip; symmetry saves 25% of the
     matmul work (elo trick) and 5 mirrored [128,128] blocks are filled by
     PE transposes at the end.
  4. Softmax via the symmetric-bias trick (see emit_post_energy): the exp
     runs directly on the PSUM E tile with the row-min as per-partition
     bias, its accumulator yields R for free, and the E bank is released at
     the exp itself — the next batch's energy stream is gated on exactly
     this. exp argument <= 0 (no overflow), R >= 1 (safe reciprocal).
  5. The value matmul att @ q runs in fp8e4 DoubleRow perf mode (2 k-tiles
     per pass, 0.5 cycles/row) against a fp8 copy of q produced on the
     otherwise-idle gpsimd engine during the load phase. gamma/R_c and the
     +x residual are fused into one DVE scalar_tensor_tensor per 512-column
     chunk (with some chunks routed ACT scale-copy + gpsimd add in the final
     tail to double production rate); x enters only here, so for gamma == 0
     the attention path cannot perturb the output beyond f32r rounding.
  6. Batch 1's energy stream is interleaved (via emission order) with batch
     0's value phase, and batch 1's softmax tiles pipeline straight into
     their output pairs, so the output DMA tail starts as early as the PE
     work total allows.
"""

import sys

import numpy as np

_REPO = "/opt/trn_rl_repo"
if _REPO not in sys.path:
    sys.path.insert(0, _REPO)

B_TOTAL, C, H, W = 16, 512, 64, 64
N = H * W          # 4096
NCORES = 8
B = B_TOTAL // NCORES  # batches per core = 2
CT = C // 128      # 4 c-tiles
NT = N // 128      # 32 n-tiles
NCH = N // 512     # 8 output column chunks
NG = N // 1024     # 4 input DMA column groups

_cache = {}


def _build_program():
    import concourse.bass as bass
    import concourse.bacc as bacc
    import concourse.mybir as mybir
    import concourse.tile as tile
    from contextlib import ExitStack

    f32 = mybir.dt.float32
    f32r = mybir.dt.float32r
    fp8 = mybir.dt.float8e4
    AX = mybir.AxisListType
    OP = mybir.AluOpType
    ACT = mybir.ActivationFunctionType
    DR = mybir.MatmulPerfMode.DoubleRow

    nc = bacc.Bacc("TRN2", target_bir_lowering=False, debug=False)

    # x is declared f32r so the natural-layout q tiles can feed the PE
    # transposes at the f32r rate (1.5 cycles/row vs fp32's 2). f32r is
    # bit-identical fp32 in memory; the DMA just moves bytes.
    x = nc.dram_tensor("x", [B, C, N], f32r, kind="ExternalInput").ap()
    g128 = nc.dram_tensor("gamma128", [128, 1], f32, kind="ExternalInput").ap()
    ident_d = nc.dram_tensor("ident", [128, 128], f32, kind="ExternalInput").ap()
    identr_d = nc.dram_tensor(
        "identr", [128, 128], f32r, kind="ExternalInput"
    ).ap()
    y = nc.dram_tensor("y", [B, C, N], f32, kind="ExternalOutput").ap()

    with ExitStack() as ctx:
        tc = ctx.enter_context(tile.TileContext(nc))
        const_p = ctx.enter_context(tc.tile_pool(name="const", bufs=1))
        qnat_p = ctx.enter_context(tc.tile_pool(name="qnat", bufs=2))
        qt_p = ctx.enter_context(tc.tile_pool(name="qt", bufs=9))
        q8_p = ctx.enter_context(tc.tile_pool(name="q8", bufs=2))
        u_p = ctx.enter_context(tc.tile_pool(name="u", bufs=2))
        unat_p = ctx.enter_context(tc.tile_pool(name="unat", bufs=2))
        sm_p = ctx.enter_context(tc.tile_pool(name="sm", bufs=2))
        osb_p = ctx.enter_context(tc.tile_pool(name="osb", bufs=4))
        # PSUM: tags are separate rings. E x4 + tp x2 + O x2 = 8 banks.
        ps = ctx.enter_context(tc.tile_pool(name="ps", bufs=1, space="PSUM"))

        ident = const_p.tile([128, 128], f32, tag="ident")
        nc.sync.dma_start(ident[:], ident_d)
        identr = const_p.tile([128, 128], f32r, tag="identr")
        nc.sync.dma_start(identr[:], identr_d)
        gam = const_p.tile([128, 1], f32, tag="gam")
        nc.sync.dma_start(gam[:], g128)
        qnat0 = qnat_p.tile([128, CT, N], f32r, tag="qnat")
        nc.sync.dma_start(qnat0[:, 0, 0:512], x[0, 0:128, 0:512])

        def tp_tile(shape, name, dtype=f32):
            return ps.tile(shape, dtype, tag="tp", name=name, bufs=2)

        def o_tile(name):
            return ps.tile([128, 512], f32, tag="O", name=name, bufs=2)

        # warm the PE clock during the initial DMA wait: a long dependency-
        # free transpose chain keeps PE continuously busy through the p-state
        # ramp so the first real transposes run at full clock.
        warm = tp_tile([128, 512], "warm", f32r)
        for w in range(16):
            nc.tensor.matmul(
                warm[:, 128 * (w % 4):128 * (w % 4 + 1)],
                identr[:],
                identr[:],
                is_transpose=True,
                skip_group_check=True,
            )

        # ---- all input DMAs up front (SP queue: nothing ahead of them can
        #      block), plus the fp8 cast of q on the gpsimd engine, piecewise
        #      so it tracks the DMA stream. Batch 0's first column group is
        #      split finer so the transpose pipeline starts sooner.
        qnats, q8s = [qnat0], []
        for b in range(B):
            qnat = qnats[b] if b == 0 else qnat_p.tile(
                [128, CT, N], f32r, tag="qnat", name="qnat"
            )
            q8 = q8_p.tile([128, CT, N], fp8, tag="q8")
            for g in range(NG):
                if b == 0 and g == 0:
                    for h in range(2):
                        for t in range(CT):
                            if h == 0 and t == 0:
                                continue  # issued before the consts
                            nc.sync.dma_start(
                                qnat[:, t, 512 * h:512 * (h + 1)],
                                x[b, 128 * t:128 * (t + 1),
                                  512 * h:512 * (h + 1)],
                            )
                else:
                    for t in range(CT):
                        nc.sync.dma_start(
                            qnat[:, t, 1024 * g:1024 * (g + 1)],
                            x[b, 128 * t:128 * (t + 1),
                              1024 * g:1024 * (g + 1)],
                        )
                for t in range(CT):
                    nc.gpsimd.tensor_copy(
                        q8[:, t, 1024 * g:1024 * (g + 1)],
                        qnat[:, t, 1024 * g:1024 * (g + 1)],
                    )
            if b > 0:
                qnats.append(qnat)
            q8s.append(q8)

        elo = [0, 128, 256, 256]
        mirrors = [(0, 1), (0, 2), (1, 2), (0, 3), (1, 3)]
        Es = [None] * B
        Us = [None] * B
        scs = [[None] * CT for _ in range(B)]

        qts = [[None] * NT for _ in range(B)]

        def emit_tp(b, j):
            """Transpose chunk j (4 row-tiles) into one PSUM bank and drain
            to the qt ring (scalar engine, rounds to f32r)."""
            qnat = qnats[b]
            if j == 0:
                Es[b] = [
                    ps.tile([128, C], f32, tag="E", name=f"E{b}t{t_}", bufs=4)
                    for t_ in range(CT)
                ]
            tp = tp_tile([128, 512], f"tp{b}j{j}", f32r)
            for t in range(CT):
                nc.tensor.matmul(
                    tp[:, 128 * t:128 * (t + 1)],
                    qnat[:, t, 128 * j:128 * (j + 1)],
                    identr[:],
                    is_transpose=True,
                    skip_group_check=True,
                )
            qt = qt_p.tile([128, C], f32r, tag="qt")
            nc.scalar.copy(qt[:], tp[:])
            qts[b][j] = qt

        def emit_E(b, j):
            """Accumulate the 4 E tiles from qt chunk j."""
            E, qt = Es[b], qts[b][j]
            for t in range(CT):
                nc.tensor.matmul(
                    E[t][:, elo[t]:C],
                    qt[:, 128 * t:128 * (t + 1)],
                    qt[:, elo[t]:C],
                    start=(j == 0),
                    stop=(j == NT - 1),
                )

        def emit_post_energy(b, value_per_tile=False, filler=None):
            """Softmax via the symmetric-bias trick, per row-tile t:

            U_nat[t][c', d] = exp(rowmin_c - E[c, d]) with c = 128t + c' is
            exactly the att numerator transposed per c-tile (E symmetric, so
            the column-min of column c equals the row-min of row c, which is
            a per-PARTITION bias for the tile). The activation's accumulator
            gives R_c = sum_d U for free, and E[t]'s PSUM bank is freed right
            at the exp — unblocking the next batch's energy accumulation.
            PE then transposes U_nat into the [d, c] orientation the value
            matmul needs (16 cheap f32r transposes + 1 drain per tile)."""
            E = Es[b]
            rmins = sm_p.tile([128, CT], f32, tag="rmins")
            U = u_p.tile([128, CT, C], fp8, tag="U")
            Us[b] = U

            def tile_chain(t):
                unat = unat_p.tile(
                    [128, C], f32r, tag="unat", name=f"unat{b}{t}"
                )
                R = sm_p.tile([128, 1], f32, tag="R", name=f"R{b}{t}")
                nc.scalar.activation(
                    unat[:], E[t][:], ACT.Exp,
                    bias=rmins[:, t:t + 1], scale=-1.0, accum_out=R[:],
                )
                # R >= 1 always: the row's own min term contributes exp(0)
                rec = sm_p.tile([128, 1], f32, tag="rec", name=f"rec{b}{t}")
                nc.vector.reciprocal(rec[:], R[:])
                sc = sm_p.tile([128, 1], f32, tag=f"scale{t}")
                nc.vector.tensor_scalar_mul(sc[:], rec[:], gam[:, 0:1])
                scs[b][t] = sc
                utp = tp_tile([128, 512], f"utp{b}{t}", f32r)
                for k in range(CT):
                    nc.tensor.matmul(
                        utp[:, 128 * k:128 * (k + 1)],
                        unat[:, 128 * k:128 * (k + 1)],
                        identr[:],
                        is_transpose=True,
                        skip_group_check=True,
                    )
                nc.scalar.copy(
                    U[:, 0:CT, 128 * t:128 * (t + 1)],
                    utp[:].rearrange("p (k c) -> p k c", k=CT),
                )
                if value_per_tile == "first":
                    # prime one output pair per tile; the rest follow in the
                    # interleaved stream
                    emit_value_pair(b, t * (NCH // 2))
                elif value_per_tile:
                    # first pair of each tile takes the short all-DVE path;
                    # later pairs use the split path for throughput
                    for nh in range(NCH // 2):
                        idx = t * (NCH // 2) + nh
                        emit_value_pair(
                            b, idx, split=(nh > 0),
                            tail=(idx == 0 or idx >= NT // 2 - 2),
                        )
                if filler is not None:
                    filler()

            # tile 0 has no incoming mirror blocks: its whole chain (row-min
            # -> exp -> U slice -> first output pairs) runs ahead of the
            # mirror work, which only tiles 1-3 need
            nc.vector.tensor_reduce(
                rmins[:, 0:1], E[0][:], axis=AX.X, op=OP.min
            )
            tile_chain(0)
            for s, tt in mirrors:
                blk = sm_p.tile(
                    [128, 128], f32, tag="mirror", name=f"blk{b}{s}{tt}",
                    bufs=3,
                )
                nc.vector.tensor_copy(blk[:], E[s][:, 128 * tt:128 * (tt + 1)])
                nc.tensor.matmul(
                    E[tt][:, 128 * s:128 * (s + 1)],
                    blk[:],
                    ident[:],
                    is_transpose=True,
                    skip_group_check=True,
                )
            for t in range(1, CT):
                nc.vector.tensor_reduce(
                    rmins[:, t:t + 1], E[t][:], axis=AX.X, op=OP.min
                )
            for t in range(1, CT):
                tile_chain(t)

        def emit_value_pair(b, idx, split=False, tail=False):
            """Two 512-column value chunks (fp8 DoubleRow) + fused residual
            + one [128,1024] output DMA. idx in [0, 16).

            split=False: both residuals as DVE STTs (PSUM direct).
            split=True: chunk 1's residual goes ACT (scale-copy from PSUM)
            then gpsimd (in-place +x), doubling pair production rate when
            DVE would otherwise pace the tail."""
            m, nh = idx // (NCH // 2), idx % (NCH // 2)
            U, q8, qnat, sc = Us[b], q8s[b], qnats[b], scs[b][m]
            osb = osb_p.tile([128, 1024], f32, tag="osb")
            for i in range(2):
                n = 2 * nh + i
                On = o_tile(f"O{b}{m}{n}")
                for p in range(CT // 2):
                    nc.tensor.matmul(
                        On[:],
                        U[:, 2 * p:2 * p + 2, 128 * m:128 * (m + 1)],
                        q8[:, 2 * p:2 * p + 2, 512 * n:512 * (n + 1)],
                        start=(p == 0),
                        stop=(p == CT // 2 - 1),
                        perf_mode=DR,
                        skip_group_check=True,
                    )
                half = osb[:, 512 * i:512 * (i + 1)]
                xs = qnat[:, m, 512 * n:512 * (n + 1)]
                if split and i == 1:
                    nc.scalar.activation(half, On[:], ACT.Copy, scale=sc[:])
                    nc.gpsimd.tensor_tensor(half, half, xs, op=OP.add)
                else:
                    nc.vector.scalar_tensor_tensor(
                        half, On[:], sc[:], xs, op0=OP.mult, op1=OP.add
                    )
                if tail:
                    # last pairs ship per-chunk so the final transfer isn't
                    # held for the second chunk's residual
                    nc.sync.dma_start(
                        y[b, 128 * m:128 * (m + 1),
                          512 * n:512 * (n + 1)],
                        half,
                    )
            if not tail:
                nc.sync.dma_start(
                    y[b, 128 * m:128 * (m + 1), 1024 * nh:1024 * (nh + 1)],
                    osb[:],
                )

        # ---- software-pipelined schedule. Two pipelining levels:
        #      * within a batch's energy stream, chunk j+1's transposes are
        #        emitted before chunk j's E matmuls so the in-order PE queue
        #        never stalls on the PSUM->SBUF drain round-trip;
        #      * batch 1's energy stream is interleaved with batch 0's value
        #        phase so the PE never sits behind the (STT-paced) value
        #        chunks.
        # Lookahead-2 on both energy streams hides the PSUM->SBUF drain
        # round-trip (with lookahead-1 the E group arrives ~160ns before the
        # drain ack). Batch 0's value pairs are spaced 2-per-3 energy chunks:
        # the 2-deep O-ring ties each pair's matmuls to the previous pair's
        # (DVE) STT, so tighter spacing would stall the in-order PE queue.
        LA = 2
        for j in range(LA):
            emit_tp(0, j)
        for j in range(LA, NT + LA):
            if j < NT:
                emit_tp(0, j)
            emit_E(0, j - LA)
        # batch 1's transpose stream is threaded through batch 0's post-
        # energy as PE filler: PE would otherwise idle behind the serialized
        # ACT exp chain there.
        k_tp = 0

        def tp_filler():
            nonlocal k_tp
            for _ in range(5):
                if k_tp < NT:
                    emit_tp(1, k_tp)
                    k_tp += 1

        emit_post_energy(0, filler=tp_filler)
        k_E = 0
        next_pair = 0
        while k_E < NT:
            if k_tp < NT:
                emit_tp(1, k_tp)
                k_tp += 1
            if k_tp - k_E >= LA or k_tp >= NT:
                emit_E(1, k_E)
                k_E += 1
                if k_E % 4 != 0 and next_pair < NT // 2:
                    emit_value_pair(0, next_pair)
                    next_pair += 1
        while next_pair < NT // 2:
            emit_value_pair(0, next_pair)
            next_pair += 1
        # batch 1's post-energy pipelines per c-tile: each tile's U slice and
        # scale feed its 4 output pairs immediately, so the output stream
        # starts ~3us after the last E matmul instead of after the full
        # softmax pass.
        emit_post_energy(1, value_per_tile=True)

    nc.compile()
    return nc


def get_program():
    if "nc" not in _cache:
        _cache["nc"] = _build_program()
    return _cache["nc"]


def kernel(x, gamma):
    from concourse.bass_utils import run_bass_kernel_spmd

    nc = get_program()
    xr = np.ascontiguousarray(
        np.asarray(x, dtype=np.float32).reshape(B_TOTAL, C, N)
    )
    g = np.asarray(gamma, dtype=np.float32).reshape(1)
    g128 = np.ascontiguousarray(
        np.broadcast_to(g.reshape(1, 1), (128, 1))
    ).astype(np.float32)
    ident = np.eye(128, dtype=np.float32)
    in_maps = [
        {
            "x": xr[i * B:(i + 1) * B],
            "gamma128": g128,
            "ident": ident,
            "identr": ident,
        }
        for i in range(NCORES)
    ]
    res = run_bass_kernel_spmd(nc, in_maps, list(range(NCORES))).results
    y = np.concatenate([res[i]["y"] for i in range(NCORES)], axis=0)
    return y.reshape(B_TOTAL, C, H, W).astype(np.float32)
